# revision 9
# baseline (speedup 1.0000x reference)
"""Trainium2 Bass kernel for GroupNorm + spatial self-attention + residual.

Reference computation (B=1, C=512, H=W=64, 8 heads x 64 dim, GN groups=32):
    x = GroupNorm(hidden_states) -> tokens [N=4096, C]
    q,k,v = x @ {wq,wk,wv}.T  (per-head slices of inner=512)
    out = softmax(q k^T / 8) v   per head
    y = concat_heads(out) @ wo.T + bo + hidden_states

Distribution over 8 NeuronCores: head-parallel attention (core h owns head h;
every core reads the full input), then an AllToAll that token-shards the
attention output so core j computes the output projection + bias + residual
for tokens [512j, 512j+512) only.

Per-core device graph (SPMD, same graph on all 8 cores, per-core data differs):
  1. DMA x [512, 4096] f32 into SBUF; per-channel bn_stats/bn_aggr;
     group stats via a block-ones matmul over the partition axis;
     rstd = exp(-0.5 ln(var+eps)); per-channel affine s,b.
  2. x_norm bf16 = x*s + b  (one tensor_scalar pass per 128-channel tile).
  3. q_T,k_T [64, 4096] and v [4096, 64+1] bf16 via PE (ones column appended to
     v so the P@V matmul also yields softmax denominators for free).
  4. Flash-style attention in transposed layout: scores_T [tk=128, tq=1024]
     per tile -> ACT exp (scale=1/8 folded in) -> bf16 p tiles -> PV accumulate
     into psum [65, 1024]. No max-subtraction (scores are O(1) by construction).
  5. Unnormalized out_aug [65, 4096] (row 64 = denominators) -> DRAM -> AllToAll.
  6. Each core normalizes its received token chunk (ln/exp reciprocal + DMA
     partition-broadcast), output projection, +bo, +residual, writes its
     [512, 512] column chunk of the output.
"""

import sys

sys.path.insert(0, "/opt/trn_rl_repo")

import numpy as np

import concourse.bacc as bacc
import concourse.tile as tile
from concourse import mybir
from concourse.bass_utils import run_bass_kernel_spmd

C = 512
N = 4096
HEADS = 8
D = 64
GROUPS = 32
CPG = C // GROUPS  # 16 channels per group
EPS = 1e-5
SCALE = D ** -0.5
NCORE = 8
NT = N // NCORE  # 512 tokens per core for the output projection
TQ = 1024  # query-chunk (free dim of transposed scores)
NTQ = N // TQ  # 4
TKC = 128  # key-chunk (partition dim of transposed scores)
NTK = N // TKC  # 32
CT = C // 128  # 4 channel tiles

f32 = mybir.dt.float32
bf16 = mybir.dt.bfloat16
AF = mybir.ActivationFunctionType
ALU = mybir.AluOpType

_nc_cache = {}


def _build():
    nc = bacc.Bacc("TRN2", target_bir_lowering=False, debug=False, num_devices=NCORE)

    x_d = nc.dram_tensor("x", [C, N], f32, kind="ExternalInput")
    gamma_d = nc.dram_tensor("gamma", [C, 1], f32, kind="ExternalInput")
    beta_d = nc.dram_tensor("beta", [C, 1], f32, kind="ExternalInput")
    wqT_d = nc.dram_tensor("wqT", [C, D], f32, kind="ExternalInput")
    wkT_d = nc.dram_tensor("wkT", [C, D], f32, kind="ExternalInput")
    wvT_d = nc.dram_tensor("wvT", [C, D], f32, kind="ExternalInput")
    woT_d = nc.dram_tensor("woT", [C, C], f32, kind="ExternalInput")
    bo_d = nc.dram_tensor("bo", [C, 1], f32, kind="ExternalInput")
    resid_d = nc.dram_tensor("resid", [C, NT], f32, kind="ExternalInput")
    bones_d = nc.dram_tensor("bones", [128, 8], f32, kind="ExternalInput")
    out_d = nc.dram_tensor("out", [C, NT], f32, kind="ExternalOutput")

    with tile.TileContext(nc) as tc:
        with (
            tc.tile_pool(name="xc", bufs=1) as pxc,
            tc.tile_pool(name="xb", bufs=1) as pxb,
            tc.tile_pool(name="qk", bufs=1) as pqk,
            tc.tile_pool(name="vaug", bufs=1) as pva,
            tc.tile_pool(name="w", bufs=1) as pw,
            tc.tile_pool(name="small", bufs=1) as psm,
            tc.tile_pool(name="p", bufs=3) as pp,
            tc.tile_pool(name="post", bufs=1) as ppost,
            tc.tile_pool(name="dram", bufs=1, space="DRAM") as pdram,
        ):
            # ---------------- stage 1: load x + GroupNorm statistics ----------
            xc = [pxc.tile([128, N], f32, name=f"xc{i}") for i in range(CT)]
            stats = [psm.tile([128, 8, 6], f32, name=f"st{i}") for i in range(CT)]
            cstat = [psm.tile([128, 2], f32, name=f"cs{i}") for i in range(CT)]
            bones = psm.tile([128, 8], f32, name="bones")
            nc.sync.dma_start(bones[:, :], bones_d[:, :])
            gstats = psm.tile([GROUPS, 2], f32, name="gstats")

            with tc.tile_pool(name="ps_g", bufs=2, space="PSUM") as ps_g:
                for i in range(CT):
                    nc.sync.dma_start(xc[i][:, :], x_d[i * 128 : (i + 1) * 128, :])
                    for j in range(8):
                        nc.vector.bn_stats(
                            out=stats[i][:, j, :], in_=xc[i][:, j * 512 : (j + 1) * 512]
                        )
                    mv = psm.tile([128, 2], f32, name="mv", tag="mv", bufs=2)
                    nc.vector.bn_aggr(out=mv[:, :], in_=stats[i][:, :, :])
                    # cstat = (mean, E[x^2]) per channel
                    nc.vector.tensor_copy(cstat[i][:, 0:1], mv[:, 0:1])
                    nc.vector.tensor_mul(cstat[i][:, 1:2], mv[:, 0:1], mv[:, 0:1])
                    nc.vector.tensor_add(cstat[i][:, 1:2], cstat[i][:, 1:2], mv[:, 1:2])
                    gps = ps_g.tile([8, 2], f32, name="gps", tag="gps")
                    nc.tensor.matmul(gps[:, :], bones[:, :], cstat[i][:, :])
                    gtmp = psm.tile([8, 2], f32, name="gtmp", tag="gtmp", bufs=2)
                    nc.vector.tensor_copy(gtmp[:, :], gps[:, :])
                    nc.sync.dma_start(gstats[i * 8 : (i + 1) * 8, :], gtmp[:, :])

            # group mean/ex2 -> mean, rstd
            gm = psm.tile([GROUPS, 2], f32, name="gm")
            nc.vector.tensor_scalar_mul(gm[:, :], gstats[:, :], 1.0 / (CPG))
            vtmp = psm.tile([GROUPS, 1], f32, name="vtmp")
            nc.vector.tensor_mul(vtmp[:, :], gm[:, 0:1], gm[:, 0:1])
            varg = psm.tile([GROUPS, 1], f32, name="varg")
            nc.vector.tensor_sub(varg[:, :], gm[:, 1:2], vtmp[:, :])
            eps_sb = psm.tile([GROUPS, 1], f32, name="eps_sb")
            nc.vector.memset(eps_sb[:, :], EPS)
            lng = psm.tile([GROUPS, 1], f32, name="lng")
            nc.scalar.activation(lng[:, :], varg[:, :], AF.Ln, bias=eps_sb[:, :])
            rstd = psm.tile([GROUPS, 1], f32, name="rstd")
            nc.scalar.activation(rstd[:, :], lng[:, :], AF.Exp, scale=-0.5)
            gs2 = psm.tile([GROUPS, 2], f32, name="gs2")
            nc.vector.tensor_copy(gs2[:, 0:1], gm[:, 0:1])
            nc.vector.tensor_copy(gs2[:, 1:2], rstd[:, :])
            gdram = pdram.tile([GROUPS, 2], f32, name="gdram")
            nc.sync.dma_start(gdram[:, :], gs2[:, :])

            gamma_sb = psm.tile([128, CT], f32, name="gamma_sb")
            beta_sb = psm.tile([128, CT], f32, name="beta_sb")
            for i in range(CT):
                nc.sync.dma_start(
                    gamma_sb[:, i : i + 1], gamma_d[i * 128 : (i + 1) * 128, :]
                )
                nc.sync.dma_start(
                    beta_sb[:, i : i + 1], beta_d[i * 128 : (i + 1) * 128, :]
                )

            s_c = [psm.tile([128, 1], f32, name=f"s_c{i}") for i in range(CT)]
            b_c = [psm.tile([128, 1], f32, name=f"b_c{i}") for i in range(CT)]
            for i in range(CT):
                cb = psm.tile([128, 2], f32, name="cb", tag="cb", bufs=2)
                # expand group stats (8 groups of this tile) x16 channels each
                src = (
                    gdram[i * 8 : (i + 1) * 8, :]
                    .rearrange("a (o c) -> a o c", o=1)
                    .broadcast_to([8, CPG, 2])
                )
                nc.sync.dma_start(cb[:, :], src)
                nc.vector.tensor_mul(s_c[i][:, :], cb[:, 1:2], gamma_sb[:, i : i + 1])
                nc.vector.tensor_mul(b_c[i][:, :], cb[:, 0:1], s_c[i][:, :])
                nc.vector.tensor_sub(b_c[i][:, :], beta_sb[:, i : i + 1], b_c[i][:, :])

            # ---------------- stage 2: x_norm (bf16) --------------------------
            xb = [pxb.tile([128, N], bf16, name=f"xb{i}") for i in range(CT)]
            for i in range(CT):
                nc.vector.tensor_scalar(
                    out=xb[i][:, :],
                    in0=xc[i][:, :],
                    scalar1=s_c[i][:, :],
                    scalar2=b_c[i][:, :],
                    op0=ALU.mult,
                    op1=ALU.add,
                )

            # ---------------- stage 3: q_T, k_T, v ----------------------------
            wq_sb = [pw.tile([128, D], f32, name=f"wq{i}") for i in range(CT)]
            wk_sb = [pw.tile([128, D], f32, name=f"wk{i}") for i in range(CT)]
            wv_sb = [pw.tile([128, D], f32, name=f"wv{i}") for i in range(CT)]
            wqb = [pw.tile([128, D], bf16, name=f"wqb{i}") for i in range(CT)]
            wkb = [pw.tile([128, D], bf16, name=f"wkb{i}") for i in range(CT)]
            wvb = [pw.tile([128, D], bf16, name=f"wvb{i}") for i in range(CT)]
            for i in range(CT):
                nc.sync.dma_start(wq_sb[i][:, :], wqT_d[i * 128 : (i + 1) * 128, :])
                nc.sync.dma_start(wk_sb[i][:, :], wkT_d[i * 128 : (i + 1) * 128, :])
                nc.sync.dma_start(wv_sb[i][:, :], wvT_d[i * 128 : (i + 1) * 128, :])
                nc.vector.tensor_copy(wqb[i][:, :], wq_sb[i][:, :])
                nc.vector.tensor_copy(wkb[i][:, :], wk_sb[i][:, :])
                nc.vector.tensor_copy(wvb[i][:, :], wv_sb[i][:, :])

            kT = pqk.tile([D, N], bf16, name="kT")
            qT = pqk.tile([D, N], bf16, name="qT")
            vaug = [pva.tile([128, D + 1], bf16, name=f"va{jj}") for jj in range(NTK)]
            with (
                tc.tile_pool(name="ps_qk", bufs=2, space="PSUM") as ps_qk,
                tc.tile_pool(name="ps_v", bufs=2, space="PSUM") as ps_v,
            ):
                for j in range(N // 512):
                    kps = ps_qk.tile([D, 512], f32, name="kps", tag="qkps")
                    for i in range(CT):
                        nc.tensor.matmul(
                            kps[:, :],
                            wkb[i][:, :],
                            xb[i][:, j * 512 : (j + 1) * 512],
                            start=(i == 0),
                            stop=(i == CT - 1),
                        )
                    nc.vector.tensor_copy(kT[:, j * 512 : (j + 1) * 512], kps[:, :])
                # v in [token, d] layout with an appended ones column
                for jj in range(NTK):
                    vps = ps_v.tile([128, D], f32, name="vps", tag="vps")
                    for i in range(CT):
                        nc.tensor.matmul(
                            vps[:, :],
                            xb[i][:, jj * 128 : (jj + 1) * 128],
                            wvb[i][:, :],
                            start=(i == 0),
                            stop=(i == CT - 1),
                        )
                    nc.vector.tensor_copy(vaug[jj][:, 0:D], vps[:, :])
                    nc.vector.memset(vaug[jj][:, D : D + 1], 1.0)
                for j in range(N // 512):
                    qps = ps_qk.tile([D, 512], f32, name="qps", tag="qkps")
                    for i in range(CT):
                        nc.tensor.matmul(
                            qps[:, :],
                            wqb[i][:, :],
                            xb[i][:, j * 512 : (j + 1) * 512],
                            start=(i == 0),
                            stop=(i == CT - 1),
                        )
                    nc.vector.tensor_copy(qT[:, j * 512 : (j + 1) * 512], qps[:, :])

            # ---------------- stage 4: attention ------------------------------
            a2a_in = pdram.tile([HEADS, D + 1, NT], f32, name="a2a_in")
            with (
                tc.tile_pool(name="ps_s", bufs=2, space="PSUM") as ps_s,
                tc.tile_pool(name="ps_o", bufs=2, space="PSUM") as ps_o,
            ):
                for jq in range(NTQ):
                    ops = ps_o.tile([D + 1, TQ], f32, name="ops", tag="ops")
                    for tk in range(NTK):
                        sps = ps_s.tile([128, TQ], f32, name="sps", tag="sps")
                        for half in range(TQ // 512):
                            nc.tensor.matmul(
                                sps[:, half * 512 : (half + 1) * 512],
                                kT[:, tk * 128 : (tk + 1) * 128],
                                qT[
                                    :,
                                    jq * TQ + half * 512 : jq * TQ + (half + 1) * 512,
                                ],
                                start=True,
                                stop=True,
                            )
                        p = pp.tile([128, TQ], bf16, name="p", tag="p")
                        nc.scalar.activation(p[:, :], sps[:, :], AF.Exp, scale=SCALE)
                        for half in range(TQ // 512):
                            nc.tensor.matmul(
                                ops[:, half * 512 : (half + 1) * 512],
                                vaug[tk][:, :],
                                p[:, half * 512 : (half + 1) * 512],
                                start=(tk == 0),
                                stop=(tk == NTK - 1),
                            )
                    o_sb = pp.tile([D + 1, TQ], f32, name="o_sb", tag="o_sb", bufs=2)
                    nc.vector.tensor_copy(o_sb[:, :], ops[:, :])
                    for half in range(TQ // 512):
                        blk = jq * (TQ // 512) + half
                        nc.sync.dma_start(
                            a2a_in[blk, :, :], o_sb[:, half * 512 : (half + 1) * 512]
                        )

            # ---------------- stage 5: AllToAll -------------------------------
            a2a_out = pdram.tile([HEADS, D + 1, NT], f32, name="a2a_out")
            nc.gpsimd.collective_compute(
                "AllToAll",
                ALU.bypass,
                replica_groups=[list(range(NCORE))],
                ins=[a2a_in.opt()],
                outs=[a2a_out.opt()],
            )

            # ---------------- stage 6: normalize + output projection ----------
            den = ppost.tile([HEADS, NT], f32, name="den")
            nc.sync.dma_start(
                den[:, :],
                a2a_out[:, D, :],
            )
            dln = ppost.tile([HEADS, NT], f32, name="dln")
            nc.scalar.activation(dln[:, :], den[:, :], AF.Ln)
            drc = ppost.tile([HEADS, NT], f32, name="drc")
            nc.scalar.activation(drc[:, :], dln[:, :], AF.Exp, scale=-1.0)
            drc_dram = pdram.tile([HEADS, NT], f32, name="drc_dram")
            nc.sync.dma_start(drc_dram[:, :], drc[:, :])

            rhs_sb = [ppost.tile([128, NT], bf16, name=f"rhs{i}") for i in range(4)]
            for h in range(HEADS):
                rcv = ppost.tile([D, NT], f32, name="rcv", tag="rcv", bufs=3)
                nc.sync.dma_start(rcv[:, :], a2a_out[h, 0:D, :])
                bcr = ppost.tile([D, NT], f32, name="bcr", tag="bcr", bufs=3)
                nc.sync.dma_start(
                    bcr[:, :],
                    drc_dram[h : h + 1, :].broadcast_to([D, NT]),
                )
                nc.vector.tensor_mul(
                    rhs_sb[h // 2][(h % 2) * D : (h % 2) * D + D, :],
                    rcv[:, :],
                    bcr[:, :],
                )

            wo_sb = [ppost.tile([128, C], f32, name=f"wo{i}") for i in range(4)]
            wob = [ppost.tile([128, C], bf16, name=f"wob{i}") for i in range(4)]
            resid_sb = [ppost.tile([128, NT], f32, name=f"res{i}") for i in range(CT)]
            bo_sb = ppost.tile([128, CT], f32, name="bo_sb")
            for i in range(4):
                nc.sync.dma_start(wo_sb[i][:, :], woT_d[i * 128 : (i + 1) * 128, :])
                nc.vector.tensor_copy(wob[i][:, :], wo_sb[i][:, :])
                nc.sync.dma_start(resid_sb[i][:, :], resid_d[i * 128 : (i + 1) * 128, :])
                nc.sync.dma_start(
                    bo_sb[:, i : i + 1], bo_d[i * 128 : (i + 1) * 128, :]
                )

            with tc.tile_pool(name="ps_y", bufs=2, space="PSUM") as ps_y:
                for c in range(CT):
                    yps = ps_y.tile([128, NT], f32, name="yps", tag="yps")
                    for i in range(4):
                        nc.tensor.matmul(
                            yps[:, :],
                            wob[i][:, c * 128 : (c + 1) * 128],
                            rhs_sb[i][:, :],
                            start=(i == 0),
                            stop=(i == 3),
                        )
                    y_sb = ppost.tile([128, NT], f32, name="y_sb", tag="y_sb", bufs=2)
                    nc.scalar.activation(
                        y_sb[:, :], yps[:, :], AF.Identity, bias=bo_sb[:, c : c + 1]
                    )
                    nc.vector.tensor_add(y_sb[:, :], y_sb[:, :], resid_sb[c][:, :])
                    nc.sync.dma_start(out_d[c * 128 : (c + 1) * 128, :], y_sb[:, :])

    nc.compile()
    return nc


def get_nc():
    if "nc" not in _nc_cache:
        _nc_cache["nc"] = _build()
    return _nc_cache["nc"]


def make_in_maps(hidden_states, gn_gamma, gn_beta, wq, wk, wv, wo, bo):
    x2d = np.ascontiguousarray(
        np.asarray(hidden_states, dtype=np.float32).reshape(C, N)
    )
    gamma = np.ascontiguousarray(np.asarray(gn_gamma, np.float32).reshape(C, 1))
    beta = np.ascontiguousarray(np.asarray(gn_beta, np.float32).reshape(C, 1))
    wq = np.asarray(wq, np.float32)
    wk = np.asarray(wk, np.float32)
    wv = np.asarray(wv, np.float32)
    woT = np.ascontiguousarray(np.asarray(wo, np.float32).T)
    bo2 = np.ascontiguousarray(np.asarray(bo, np.float32).reshape(C, 1))
    bones = np.zeros((128, 8), np.float32)
    for cc in range(128):
        bones[cc, cc // CPG] = 1.0
    in_maps = []
    for h in range(NCORE):
        sl = slice(h * D, (h + 1) * D)
        in_maps.append(
            {
                "x": x2d,
                "gamma": gamma,
                "beta": beta,
                "wqT": np.ascontiguousarray(wq[sl, :].T),
                "wkT": np.ascontiguousarray(wk[sl, :].T),
                "wvT": np.ascontiguousarray(wv[sl, :].T),
                "woT": woT,
                "bo": bo2,
                "resid": np.ascontiguousarray(x2d[:, h * NT : (h + 1) * NT]),
                "bones": bones,
            }
        )
    return in_maps


def kernel(hidden_states, gn_gamma, gn_beta, wq, wk, wv, wo, bo):
    nc = get_nc()
    in_maps = make_in_maps(hidden_states, gn_gamma, gn_beta, wq, wk, wv, wo, bo)
    res = run_bass_kernel_spmd(nc, in_maps, core_ids=list(range(NCORE)))
    out2d = np.empty((C, N), np.float32)
    for h in range(NCORE):
        out2d[:, h * NT : (h + 1) * NT] = res.results[h]["out"]
    return out2d.reshape(1, C, 64, 64)


# revision 26
# speedup vs baseline: 10041.5848x; 10041.5848x over previous
"""Trainium2 Bass kernel for GroupNorm + spatial self-attention + residual.

Reference computation (B=1, C=512, H=W=64, 8 heads x 64 dim, GN groups=32):
    x = GroupNorm(hidden_states) -> tokens [N=4096, C]
    q,k,v = x @ {wq,wk,wv}.T  (per-head slices of inner=512)
    out = softmax(q k^T / 8) v   per head
    y = concat_heads(out) @ wo.T + bo + hidden_states

Distribution over 8 NeuronCores: head-parallel attention (core h owns head h;
every core reads the full input), then an AllToAll that token-shards the
attention output so core j computes the output projection + bias + residual
for tokens [512j, 512j+512) only.

Per-core device graph (SPMD, same graph on all 8 cores, per-core data differs):
  1. DMA x [512, 4096] f32 into SBUF; per-channel bn_stats/bn_aggr;
     group stats via a block-ones matmul over the partition axis;
     rstd = exp(-0.5 ln(var+eps)); per-channel affine s,b.
  2. x_norm bf16 = x*s + b  (one tensor_scalar pass per 128-channel tile).
  3. q_T,k_T [64, 4096] and v [4096, 64+1] bf16 via PE (ones column appended to
     v so the P@V matmul also yields softmax denominators for free).
  4. Flash-style attention in transposed layout: scores_T [tk=128, tq=1024]
     per tile -> ACT exp (scale=1/8 folded in) -> bf16 p tiles -> PV accumulate
     into psum [65, 1024]. No max-subtraction (scores are O(1) by construction).
  5. Unnormalized out_aug [65, 4096] (row 64 = denominators) -> DRAM -> AllToAll.
  6. Each core normalizes its received token chunk (ln/exp reciprocal + DMA
     partition-broadcast), output projection, +bo, +residual, writes its
     [512, 512] column chunk of the output.
"""

import sys

sys.path.insert(0, "/opt/trn_rl_repo")

import numpy as np

import concourse.bacc as bacc
import concourse.tile as tile
from concourse import mybir
from concourse.bass_utils import run_bass_kernel_spmd

C = 512
N = 4096
HEADS = 8
D = 64
GROUPS = 32
CPG = C // GROUPS  # 16 channels per group
EPS = 1e-5
SCALE = D ** -0.5
NCORE = 8
NT = N // NCORE  # 512 tokens per core for the output projection
TQ = 1024  # query-chunk (free dim of transposed scores)
NTQ = N // TQ  # 4
TKC = 128  # key-chunk (partition dim of transposed scores)
NTK = N // TKC  # 32
CT = C // 128  # 4 channel tiles

f32 = mybir.dt.float32
bf16 = mybir.dt.bfloat16
AF = mybir.ActivationFunctionType
ALU = mybir.AluOpType

_nc_cache = {}

# exp(SCALE*x) ~= ((x*EC0 + EC1)^2 + 0.5)^16  -- a (1 + y/16 + y^2/512)^16
# approximation computed in one fused VectorE pass (8 ALU stages), used to
# split softmax exp work between ScalarE and VectorE. Max rel err 2.9e-3 at
# |y|=1.6 (scores here stay well inside that), 3.5e-4 for |y|<0.8.
EC0 = SCALE / float(np.sqrt(512.0))
EC1 = float(np.sqrt(0.5))
EXP_DVE_PAT = (False, True)  # per-tk cycle: True -> VectorE exp16, False -> ScalarE
SKEW = 1  # scores/exp run this many tk-steps ahead of PV
EXP_MODE = "split"  # "split" | "colsplit" | "act" | "copy"(diagnostic)
EXP_COLSPLIT = 384  # colsplit mode: ScalarE does [0,cs), VectorE [cs,TQ)


def _register_exp16():
    from concourse import dve_ops as dops
    from concourse.dve_spec import Spec, Src0, C0, C1, sq

    for op in dops.OPS:
        if op.name == "EXP16_ANT":
            return op
    t = sq(Src0 * C0 + C1) + C2_LEAF
    body = sq(sq(sq(sq(t))))
    spec = Spec(
        body=body,
        reference=lambda in0, in1, s0, s1, imm2: ((in0 * s0 + s1) ** 2 + imm2)
        ** 16,
    )
    op = dops.DveOp("EXP16_ANT", spec, subdim=False, uops_sha={})
    dops.OPS.append(op)
    dops.CUSTOM_DVE_SPECS[op.name] = op.spec
    dops._SUB_OPCODE_FOR_NAME[op.name] = dops._CUSTOM_DVE_ROW_BASE + len(dops.OPS) - 1
    from concourse.dve_uop import DveOpSpec
    from concourse.dve_spec import lower as dve_lower

    for ver in ("v3", "v4"):
        try:
            uops = dve_lower(spec, ver=ver)
            sha = DveOpSpec(
                name=op.name,
                opcode=dops.get_dve_sub_opcode(op.name),
                uops=uops,
                rd1_en=False,
            ).sha(ver)
            op.uops_sha[ver] = sha
        except Exception:
            pass
    return op


from concourse.dve_spec import C2 as C2_LEAF  # noqa: E402

EXP16 = _register_exp16()


def _attention_stage(nc, tc, ps_s, ps_o, pp, kT, qT, vaug, a2a_in):
    for jq in range(NTQ):
        ops = ps_o.tile([D + 1, TQ], f32, name="ops", tag="ops")
        p_tiles = {}

        def mm_scores(tk):
            sps = ps_s.tile([128, TQ], f32, name="sps", tag="sps", bufs=3)
            for half in range(TQ // 512):
                nc.tensor.matmul(
                    sps[:, half * 512 : (half + 1) * 512],
                    kT[:, tk * 128 : (tk + 1) * 128],
                    qT[:, jq * TQ + half * 512 : jq * TQ + (half + 1) * 512],
                    start=True,
                    stop=True,
                )
            p = pp.tile([128, TQ], bf16, name="p", tag="p", bufs=5)
            if EXP_MODE == "copy":
                nc.scalar.activation(p[:, :], sps[:, :], AF.Copy, scale=SCALE)
            elif EXP_MODE == "colsplit":
                cs = EXP_COLSPLIT
                nc.scalar.activation(
                    p[:, 0:cs], sps[:, 0:cs], AF.Exp, scale=SCALE
                )
                nc.vector._custom_dve(
                    EXP16, out=p[:, cs:TQ], in0=sps[:, cs:TQ],
                    s0=EC0, s1=EC1, imm2=0.5,
                )
            elif EXP_MODE == "split" and EXP_DVE_PAT[tk % len(EXP_DVE_PAT)]:
                nc.vector._custom_dve(
                    EXP16, out=p[:, :], in0=sps[:, :], s0=EC0, s1=EC1, imm2=0.5
                )
            else:
                nc.scalar.activation(p[:, :], sps[:, :], AF.Exp, scale=SCALE)
            p_tiles[tk] = p

        def mm_pv(tk):
            p = p_tiles.pop(tk)
            for half in range(TQ // 512):
                nc.tensor.matmul(
                    ops[:, half * 512 : (half + 1) * 512],
                    vaug[tk][:, :],
                    p[:, half * 512 : (half + 1) * 512],
                    start=(tk == 0),
                    stop=(tk == NTK - 1),
                )

        # software pipeline: scores/exp run SKEW tk-steps ahead of the PV
        # accumulation so the PE never waits on a freshly issued exp.
        for tk in range(SKEW):
            mm_scores(tk)
        for tk in range(SKEW, NTK):
            mm_scores(tk)
            mm_pv(tk - SKEW)
        for tk in range(NTK - SKEW, NTK):
            mm_pv(tk)

        o_sb = pp.tile([D + 1, TQ], f32, name="o_sb", tag="o_sb", bufs=2)
        nc.vector.tensor_copy(o_sb[:, :], ops[:, :])
        for half in range(TQ // 512):
            blk = jq * (TQ // 512) + half
            nc.sync.dma_start(
                a2a_in[blk, :, :], o_sb[:, half * 512 : (half + 1) * 512]
            )


def _build(attn_loop_k=None, pre_loop_k=None):
    import contextlib
    nc = bacc.Bacc("TRN2", target_bir_lowering=False, debug=False, num_devices=NCORE)

    x_d = nc.dram_tensor("x", [C, N], f32, kind="ExternalInput")
    gamma_d = nc.dram_tensor("gamma", [C, 1], f32, kind="ExternalInput")
    beta_d = nc.dram_tensor("beta", [C, 1], f32, kind="ExternalInput")
    wqT_d = nc.dram_tensor("wqT", [C, D], f32, kind="ExternalInput")
    wkT_d = nc.dram_tensor("wkT", [C, D], f32, kind="ExternalInput")
    wvT_d = nc.dram_tensor("wvT", [C, D], f32, kind="ExternalInput")
    woT_d = nc.dram_tensor("woT", [C, C], f32, kind="ExternalInput")
    bo_d = nc.dram_tensor("bo", [C, 1], f32, kind="ExternalInput")
    resid_d = nc.dram_tensor("resid", [C, NT], f32, kind="ExternalInput")
    bones_d = nc.dram_tensor("bones", [128, 8], f32, kind="ExternalInput")
    out_d = nc.dram_tensor("out", [C, NT], f32, kind="ExternalOutput")

    with tile.TileContext(nc) as tc:
        with (
            tc.tile_pool(name="xc", bufs=1) as pxc,
            tc.tile_pool(name="xb", bufs=1) as pxb,
            tc.tile_pool(name="qk", bufs=1) as pqk,
            tc.tile_pool(name="vaug", bufs=1) as pva,
            tc.tile_pool(name="w", bufs=1) as pw,
            tc.tile_pool(name="small", bufs=1) as psm,
            tc.tile_pool(name="p", bufs=3) as pp,
            tc.tile_pool(name="post", bufs=1) as ppost,
            tc.tile_pool(name="dram", bufs=1, space="DRAM") as pdram,
        ):
            # ---------------- stage 1: load x + GroupNorm statistics ----------
            pre_cm = (
                tc.For_i(0, pre_loop_k, 1)
                if pre_loop_k
                else contextlib.nullcontext()
            )
            pre_cm.__enter__()
            xc = [pxc.tile([128, N], f32, name=f"xc{i}") for i in range(CT)]
            stats = [psm.tile([128, 8, 6], f32, name=f"st{i}") for i in range(CT)]
            cstat = [psm.tile([128, 2], f32, name=f"cs{i}") for i in range(CT)]
            bones = psm.tile([128, 8], f32, name="bones")
            nc.sync.dma_start(bones[:, :], bones_d[:, :])
            gstats = psm.tile([GROUPS, 2], f32, name="gstats")

            with tc.tile_pool(name="ps_g", bufs=2, space="PSUM") as ps_g:
                for i in range(CT):
                    nc.sync.dma_start(xc[i][:, :], x_d[i * 128 : (i + 1) * 128, :])
                    for j in range(8):
                        nc.vector.bn_stats(
                            out=stats[i][:, j, :], in_=xc[i][:, j * 512 : (j + 1) * 512]
                        )
                    mv = psm.tile([128, 2], f32, name="mv", tag="mv", bufs=2)
                    nc.vector.bn_aggr(out=mv[:, :], in_=stats[i][:, :, :])
                    # cstat = (mean, E[x^2]) per channel
                    nc.vector.tensor_copy(cstat[i][:, 0:1], mv[:, 0:1])
                    nc.vector.tensor_mul(cstat[i][:, 1:2], mv[:, 0:1], mv[:, 0:1])
                    nc.vector.tensor_add(cstat[i][:, 1:2], cstat[i][:, 1:2], mv[:, 1:2])
                    gps = ps_g.tile([8, 2], f32, name="gps", tag="gps")
                    nc.tensor.matmul(gps[:, :], bones[:, :], cstat[i][:, :])
                    gtmp = psm.tile([8, 2], f32, name="gtmp", tag="gtmp", bufs=2)
                    nc.vector.tensor_copy(gtmp[:, :], gps[:, :])
                    nc.sync.dma_start(gstats[i * 8 : (i + 1) * 8, :], gtmp[:, :])

            # group mean/ex2 -> mean, rstd
            gm = psm.tile([GROUPS, 2], f32, name="gm")
            nc.vector.tensor_scalar_mul(gm[:, :], gstats[:, :], 1.0 / (CPG))
            vtmp = psm.tile([GROUPS, 1], f32, name="vtmp")
            nc.vector.tensor_mul(vtmp[:, :], gm[:, 0:1], gm[:, 0:1])
            varg = psm.tile([GROUPS, 1], f32, name="varg")
            nc.vector.tensor_sub(varg[:, :], gm[:, 1:2], vtmp[:, :])
            eps_sb = psm.tile([GROUPS, 1], f32, name="eps_sb")
            nc.vector.memset(eps_sb[:, :], EPS)
            lng = psm.tile([GROUPS, 1], f32, name="lng")
            nc.scalar.activation(lng[:, :], varg[:, :], AF.Ln, bias=eps_sb[:, :])
            rstd = psm.tile([GROUPS, 1], f32, name="rstd")
            nc.scalar.activation(rstd[:, :], lng[:, :], AF.Exp, scale=-0.5)
            gs2 = psm.tile([GROUPS, 2], f32, name="gs2")
            nc.vector.tensor_copy(gs2[:, 0:1], gm[:, 0:1])
            nc.vector.tensor_copy(gs2[:, 1:2], rstd[:, :])
            gdram = pdram.tile([GROUPS, 2], f32, name="gdram")
            nc.sync.dma_start(gdram[:, :], gs2[:, :])

            gamma_sb = psm.tile([128, CT], f32, name="gamma_sb")
            beta_sb = psm.tile([128, CT], f32, name="beta_sb")
            for i in range(CT):
                nc.sync.dma_start(
                    gamma_sb[:, i : i + 1], gamma_d[i * 128 : (i + 1) * 128, :]
                )
                nc.sync.dma_start(
                    beta_sb[:, i : i + 1], beta_d[i * 128 : (i + 1) * 128, :]
                )

            s_c = [psm.tile([128, 1], f32, name=f"s_c{i}") for i in range(CT)]
            b_c = [psm.tile([128, 1], f32, name=f"b_c{i}") for i in range(CT)]
            for i in range(CT):
                cb = psm.tile([128, 2], f32, name="cb", tag="cb", bufs=2)
                # expand group stats (8 groups of this tile) x16 channels each
                src = (
                    gdram[i * 8 : (i + 1) * 8, :]
                    .rearrange("a (o c) -> a o c", o=1)
                    .broadcast_to([8, CPG, 2])
                )
                nc.sync.dma_start(cb[:, :], src)
                nc.vector.tensor_mul(s_c[i][:, :], cb[:, 1:2], gamma_sb[:, i : i + 1])
                nc.vector.tensor_mul(b_c[i][:, :], cb[:, 0:1], s_c[i][:, :])
                nc.vector.tensor_sub(b_c[i][:, :], beta_sb[:, i : i + 1], b_c[i][:, :])

            # ---------------- stage 2: x_norm (bf16) --------------------------
            xb = [pxb.tile([128, N], bf16, name=f"xb{i}") for i in range(CT)]
            for i in range(CT):
                nc.vector.tensor_scalar(
                    out=xb[i][:, :],
                    in0=xc[i][:, :],
                    scalar1=s_c[i][:, :],
                    scalar2=b_c[i][:, :],
                    op0=ALU.mult,
                    op1=ALU.add,
                )

            # ---------------- stage 3: q_T, k_T, v ----------------------------
            wq_sb = [pw.tile([128, D], f32, name=f"wq{i}") for i in range(CT)]
            wk_sb = [pw.tile([128, D], f32, name=f"wk{i}") for i in range(CT)]
            wv_sb = [pw.tile([128, D], f32, name=f"wv{i}") for i in range(CT)]
            wqb = [pw.tile([128, D], bf16, name=f"wqb{i}") for i in range(CT)]
            wkb = [pw.tile([128, D], bf16, name=f"wkb{i}") for i in range(CT)]
            wvb = [pw.tile([128, D], bf16, name=f"wvb{i}") for i in range(CT)]
            for i in range(CT):
                nc.sync.dma_start(wq_sb[i][:, :], wqT_d[i * 128 : (i + 1) * 128, :])
                nc.sync.dma_start(wk_sb[i][:, :], wkT_d[i * 128 : (i + 1) * 128, :])
                nc.sync.dma_start(wv_sb[i][:, :], wvT_d[i * 128 : (i + 1) * 128, :])
                nc.vector.tensor_copy(wqb[i][:, :], wq_sb[i][:, :])
                nc.vector.tensor_copy(wkb[i][:, :], wk_sb[i][:, :])
                nc.vector.tensor_copy(wvb[i][:, :], wv_sb[i][:, :])

            kT = pqk.tile([D, N], bf16, name="kT")
            qT = pqk.tile([D, N], bf16, name="qT")
            vaug = [pva.tile([128, D + 1], bf16, name=f"va{jj}") for jj in range(NTK)]
            with (
                tc.tile_pool(name="ps_qk", bufs=2, space="PSUM") as ps_qk,
                tc.tile_pool(name="ps_v", bufs=2, space="PSUM") as ps_v,
            ):
                for j in range(N // 512):
                    kps = ps_qk.tile([D, 512], f32, name="kps", tag="qkps")
                    for i in range(CT):
                        nc.tensor.matmul(
                            kps[:, :],
                            wkb[i][:, :],
                            xb[i][:, j * 512 : (j + 1) * 512],
                            start=(i == 0),
                            stop=(i == CT - 1),
                        )
                    nc.vector.tensor_copy(kT[:, j * 512 : (j + 1) * 512], kps[:, :])
                # v in [token, d] layout with an appended ones column
                for jj in range(NTK):
                    vps = ps_v.tile([128, D], f32, name="vps", tag="vps")
                    for i in range(CT):
                        nc.tensor.matmul(
                            vps[:, :],
                            xb[i][:, jj * 128 : (jj + 1) * 128],
                            wvb[i][:, :],
                            start=(i == 0),
                            stop=(i == CT - 1),
                        )
                    nc.vector.tensor_copy(vaug[jj][:, 0:D], vps[:, :])
                    nc.vector.memset(vaug[jj][:, D : D + 1], 1.0)
                for j in range(N // 512):
                    qps = ps_qk.tile([D, 512], f32, name="qps", tag="qkps")
                    for i in range(CT):
                        nc.tensor.matmul(
                            qps[:, :],
                            wqb[i][:, :],
                            xb[i][:, j * 512 : (j + 1) * 512],
                            start=(i == 0),
                            stop=(i == CT - 1),
                        )
                    nc.vector.tensor_copy(qT[:, j * 512 : (j + 1) * 512], qps[:, :])

            pre_cm.__exit__(None, None, None)

            # ---------------- stage 4: attention ------------------------------
            a2a_in = pdram.tile([HEADS, D + 1, NT], f32, name="a2a_in")
            with (
                tc.tile_pool(name="ps_s", bufs=3, space="PSUM") as ps_s,
                tc.tile_pool(name="ps_o", bufs=1, space="PSUM") as ps_o,
            ):
                import contextlib

                loop_cm = (
                    tc.For_i(
                        0,
                        attn_loop_k,
                        1,
                        hint_engines=(
                            mybir.EngineType.PE,
                            mybir.EngineType.Activation,
                        ),
                    )
                    if attn_loop_k
                    else contextlib.nullcontext()
                )
                with loop_cm:
                    _attention_stage(nc, tc, ps_s, ps_o, pp, kT, qT, vaug, a2a_in)

            # ---------------- stage 5: AllToAll -------------------------------
            a2a_out = pdram.tile([HEADS, D + 1, NT], f32, name="a2a_out")
            nc.gpsimd.collective_compute(
                "AllToAll",
                ALU.bypass,
                replica_groups=[list(range(NCORE))],
                ins=[a2a_in.opt()],
                outs=[a2a_out.opt()],
            )

            # ---------------- stage 6: normalize + output projection ----------
            den = ppost.tile([HEADS, NT], f32, name="den")
            nc.sync.dma_start(
                den[:, :],
                a2a_out[:, D, :],
            )
            dln = ppost.tile([HEADS, NT], f32, name="dln")
            nc.scalar.activation(dln[:, :], den[:, :], AF.Ln)
            drc = ppost.tile([HEADS, NT], f32, name="drc")
            nc.scalar.activation(drc[:, :], dln[:, :], AF.Exp, scale=-1.0)
            drc_dram = pdram.tile([HEADS, NT], f32, name="drc_dram")
            nc.sync.dma_start(drc_dram[:, :], drc[:, :])

            rhs_sb = [ppost.tile([128, NT], bf16, name=f"rhs{i}") for i in range(4)]
            for h in range(HEADS):
                rcv = ppost.tile([D, NT], f32, name="rcv", tag="rcv", bufs=3)
                nc.sync.dma_start(rcv[:, :], a2a_out[h, 0:D, :])
                bcr = ppost.tile([D, NT], f32, name="bcr", tag="bcr", bufs=3)
                nc.sync.dma_start(
                    bcr[:, :],
                    drc_dram[h : h + 1, :].broadcast_to([D, NT]),
                )
                nc.vector.tensor_mul(
                    rhs_sb[h // 2][(h % 2) * D : (h % 2) * D + D, :],
                    rcv[:, :],
                    bcr[:, :],
                )

            wo_sb = [ppost.tile([128, C], f32, name=f"wo{i}") for i in range(4)]
            wob = [ppost.tile([128, C], bf16, name=f"wob{i}") for i in range(4)]
            resid_sb = [ppost.tile([128, NT], f32, name=f"res{i}") for i in range(CT)]
            bo_sb = ppost.tile([128, CT], f32, name="bo_sb")
            for i in range(4):
                nc.sync.dma_start(wo_sb[i][:, :], woT_d[i * 128 : (i + 1) * 128, :])
                nc.vector.tensor_copy(wob[i][:, :], wo_sb[i][:, :])
                nc.sync.dma_start(resid_sb[i][:, :], resid_d[i * 128 : (i + 1) * 128, :])
                nc.sync.dma_start(
                    bo_sb[:, i : i + 1], bo_d[i * 128 : (i + 1) * 128, :]
                )

            with tc.tile_pool(name="ps_y", bufs=2, space="PSUM") as ps_y:
                for c in range(CT):
                    yps = ps_y.tile([128, NT], f32, name="yps", tag="yps")
                    for i in range(4):
                        nc.tensor.matmul(
                            yps[:, :],
                            wob[i][:, c * 128 : (c + 1) * 128],
                            rhs_sb[i][:, :],
                            start=(i == 0),
                            stop=(i == 3),
                        )
                    y_sb = ppost.tile([128, NT], f32, name="y_sb", tag="y_sb", bufs=2)
                    nc.scalar.activation(
                        y_sb[:, :], yps[:, :], AF.Identity, bias=bo_sb[:, c : c + 1]
                    )
                    nc.vector.tensor_add(y_sb[:, :], y_sb[:, :], resid_sb[c][:, :])
                    nc.sync.dma_start(out_d[c * 128 : (c + 1) * 128, :], y_sb[:, :])

    nc.compile()
    return nc


def get_nc():
    if "nc" not in _nc_cache:
        _nc_cache["nc"] = _build()
    return _nc_cache["nc"]


def make_in_maps(hidden_states, gn_gamma, gn_beta, wq, wk, wv, wo, bo):
    x2d = np.ascontiguousarray(
        np.asarray(hidden_states, dtype=np.float32).reshape(C, N)
    )
    gamma = np.ascontiguousarray(np.asarray(gn_gamma, np.float32).reshape(C, 1))
    beta = np.ascontiguousarray(np.asarray(gn_beta, np.float32).reshape(C, 1))
    wq = np.asarray(wq, np.float32)
    wk = np.asarray(wk, np.float32)
    wv = np.asarray(wv, np.float32)
    woT = np.ascontiguousarray(np.asarray(wo, np.float32).T)
    bo2 = np.ascontiguousarray(np.asarray(bo, np.float32).reshape(C, 1))
    bones = np.zeros((128, 8), np.float32)
    for cc in range(128):
        bones[cc, cc // CPG] = 1.0
    in_maps = []
    for h in range(NCORE):
        sl = slice(h * D, (h + 1) * D)
        in_maps.append(
            {
                "x": x2d,
                "gamma": gamma,
                "beta": beta,
                "wqT": np.ascontiguousarray(wq[sl, :].T),
                "wkT": np.ascontiguousarray(wk[sl, :].T),
                "wvT": np.ascontiguousarray(wv[sl, :].T),
                "woT": woT,
                "bo": bo2,
                "resid": np.ascontiguousarray(x2d[:, h * NT : (h + 1) * NT]),
                "bones": bones,
            }
        )
    return in_maps


def kernel(hidden_states, gn_gamma, gn_beta, wq, wk, wv, wo, bo):
    nc = get_nc()
    in_maps = make_in_maps(hidden_states, gn_gamma, gn_beta, wq, wk, wv, wo, bo)
    res = run_bass_kernel_spmd(nc, in_maps, core_ids=list(range(NCORE)))
    out2d = np.empty((C, N), np.float32)
    for h in range(NCORE):
        out2d[:, h * NT : (h + 1) * NT] = res.results[h]["out"]
    return out2d.reshape(1, C, 64, 64)
